# revision 12
# baseline (speedup 1.0000x reference)
"""Trainium2 Bass kernel for nn_Net_91268055040039 (dense_mlp).

Computes out[b] = sum_{t,p} x[b,t,p] * |W[t,p]| * fc1_w[0, t*P+p] + fc1_b
  x: [32, 400, 10000] f32, W: [400, 10000] f32, fc1_w: [1, 4000000] f32.

Strategy: shard the reduction dim T=400 into 8 slices of 50 rows (this moves
64MB of x + only 4MB of params per core, vs 64+32MB for batch sharding).
Per core:
  v = |W_shard| * fc1_shard                       (one fused DVE op)
  for b in 32: acc[:, b] = reduce_add(x_tile * v) (one fused DVE op per batch,
                                                   tile layout [125 part, 4000 free])
  psum[1, 32] = ones[125,1].T @ acc[125,32]       (PE partition reduction)
Host sums the 8 per-core partials and adds fc1_b.
"""

import numpy as np

import concourse.bass as bass
import concourse.bacc as bacc
import concourse.mybir as mybir
from concourse.tile import TileContext
from concourse.bass_utils import run_bass_kernel_spmd

B, T, P = 32, 400, 10000
NCORES = 8
TS = T // NCORES          # 50 T-rows per core
K = TS * P                # 500000 reduction elements per core per batch
PART = 125                # SBUF partitions used (125*4000 == 500000 exactly)
FREE = K // PART          # 4000
F32 = mybir.dt.float32

# Set by test harness to capture an NTFF profile; harmless when False.
TRACE = False
LAST_RESULT = None


def build_program() -> bass.Bass:
    # Bacc (not raw Bass): its compile() splits multi-sem waits into separate
    # instructions — this neuronxcc build allows only 1 sync-wait per inst.
    nc = bacc.Bacc()
    xs = nc.declare_dram_parameter("xs", [B, PART, FREE], F32, isOutput=False)
    # wf[:, :FREE] = W shard, wf[:, FREE:] = fc1 shard — one DMA for both, so
    # the fused abs*mult below has a single sync-wait (walrus STT limit).
    wf = nc.declare_dram_parameter("wf", [PART, 2 * FREE], F32, isOutput=False)
    out = nc.declare_dram_parameter("out", [1, B], F32, isOutput=True)

    with TileContext(nc) as tc:
        with (
            tc.tile_pool(name="const", bufs=1) as cpool,
            tc.tile_pool(name="xp", bufs=6) as xpool,
            tc.tile_pool(name="psum", bufs=1, space="PSUM") as ppool,
        ):
            wft = cpool.tile([PART, 2 * FREE], F32)
            nc.gpsimd.dma_start(out=wft, in_=wf[:, :])
            # v = |W| * fc1: abs on the scalar engine, multiply on DVE.
            absw = cpool.tile([PART, FREE], F32)
            nc.scalar.activation(
                out=absw,
                in_=wft[:, :FREE],
                func=mybir.ActivationFunctionType.Abs,
            )
            v = cpool.tile([PART, FREE], F32)
            nc.vector.tensor_tensor(
                out=v,
                in0=absw,
                in1=wft[:, FREE:],
                op=mybir.AluOpType.mult,
            )

            ones = cpool.tile([PART, 1], F32)
            nc.vector.memset(ones, 1.0)
            acc = cpool.tile([PART, B], F32)
            scratch = cpool.tile([PART, FREE], F32)

            # Each DMA path is rate-limited on its own (HWDGE descriptor-gen
            # ~136 GB/s per ring, SWDGE ~162 GB/s measured), so spread the
            # batch loads across all three issuing engines to stack them.
            dma_engines = [nc.sync, nc.scalar, nc.gpsimd]
            for b in range(B):
                xt = xpool.tile([PART, FREE], F32, tag="xt")
                dma_engines[b % 3].dma_start(out=xt, in_=xs[b])
                # Fused multiply + free-dim reduce in one DVE pass:
                # scratch = (xt bypass 0) mult v;  acc[:, b] = sum(scratch).
                # (tensor_tensor_reduce crashes this HW/runtime build.)
                nc.vector.scalar_tensor_tensor(
                    out=scratch,
                    in0=xt,
                    scalar=0.0,
                    in1=v,
                    op0=mybir.AluOpType.bypass,
                    op1=mybir.AluOpType.mult,
                    accum_out=acc[:, b : b + 1],
                )

            ps = ppool.tile([1, B], F32)
            nc.tensor.matmul(out=ps, lhsT=ones, rhs=acc, start=True, stop=True)
            res = cpool.tile([1, B], F32)
            nc.scalar.copy(res, ps)
            nc.sync.dma_start(out=out[:, :], in_=res)
    nc.finalize()
    return nc


def make_in_maps(x: np.ndarray, W: np.ndarray, fc1_w: np.ndarray):
    x = np.asarray(x, dtype=np.float32)
    W = np.asarray(W, dtype=np.float32)
    fc1_w = np.asarray(fc1_w, dtype=np.float32)
    fc1_flat = fc1_w.reshape(T, P)
    in_maps = []
    for c in range(NCORES):
        t0 = c * TS
        xs = np.ascontiguousarray(x[:, t0 : t0 + TS, :]).reshape(B, PART, FREE)
        ws = np.ascontiguousarray(W[t0 : t0 + TS, :]).reshape(PART, FREE)
        fs = np.ascontiguousarray(fc1_flat[t0 : t0 + TS, :]).reshape(PART, FREE)
        in_maps.append({"xs": xs, "wf": np.concatenate([ws, fs], axis=1)})
    return in_maps


def kernel(x, W, fc1_w, fc1_b):
    global LAST_RESULT
    nc = build_program()
    in_maps = make_in_maps(x, W, fc1_w)
    res = run_bass_kernel_spmd(
        nc, in_maps, core_ids=list(range(NCORES)), trace=TRACE
    )
    LAST_RESULT = res
    partial = np.zeros(B, dtype=np.float64)
    for r in res.results:
        partial += r["out"][0].astype(np.float64)
    out = partial.astype(np.float32) + np.float32(np.asarray(fc1_b).reshape(-1)[0])
    return out.reshape(B, 1).astype(np.float32)


# revision 13
# speedup vs baseline: 2.6056x; 2.6056x over previous
"""Trainium2 Bass kernel for nn_Net_91268055040039 (dense_mlp).

Computes out[b] = sum_{t,p} x[b,t,p] * |W[t,p]| * fc1_w[0, t*P+p] + fc1_b
  x: [32, 400, 10000] f32, W: [400, 10000] f32, fc1_w: [1, 4000000] f32.

Strategy: shard the reduction dim T=400 into 8 slices of 50 rows (64MB of x +
4MB of params per core, vs 64+32MB for batch sharding). Per core the 500000
reduction elements per batch are padded to 128*3907 and laid out
partition-major ON THE HOST, so each SBUF partition's data for consecutive
batches is contiguous in HBM. DMA then moves 8MB chunks with 62.5KB
contiguous per-partition runs (~397 GB/s measured on this setup, vs 181 GB/s
for 16KB runs - descriptor overhead dominates short runs).

Per core:
  v = |W_shard| * fc1_shard              (ACT abs + DVE mult, in-place)
  for b in 32: acc[:, b] = reduce_add(x_tile_b * v)   (one fused DVE
        scalar_tensor_tensor with accum_out per batch; tensor_tensor_reduce
        crashes this HW/runtime build)
  psum[1, 32] = ones[128,1].T @ acc[128,32]           (PE partition reduction)
Host sums the 8 per-core partials and adds fc1_b.
"""

import numpy as np

import concourse.bass as bass
import concourse.bacc as bacc
import concourse.mybir as mybir
from concourse.tile import TileContext
from concourse.bass_utils import run_bass_kernel_spmd

B, T, P = 32, 400, 10000
NCORES = 8
TS = T // NCORES          # 50 T-rows per core
K = TS * P                # 500000 reduction elements per core per batch
PART = 128
FREE = 3907               # ceil(K / PART); 128*3907 = 500096 (96 zero pad)
KPAD = PART * FREE
CHUNK = 4                 # batches per DMA: 4 * 3907 * 4B = 62.5KB per row
NCHUNKS = B // CHUNK
F32 = mybir.dt.float32

# Set by the test harness to capture an NTFF profile; harmless when False.
TRACE = False
LAST_RESULT = None


def build_program() -> bass.Bass:
    # Bacc (not raw Bass): its compile() splits multi-sem waits into separate
    # instructions - this neuronxcc build allows only 1 sync-wait per inst.
    nc = bacc.Bacc()
    xs = nc.declare_dram_parameter("xs", [PART, B * FREE], F32, isOutput=False)
    # wf[:, :FREE] = W shard, wf[:, FREE:] = fc1 shard (one DMA for both).
    wf = nc.declare_dram_parameter("wf", [PART, 2 * FREE], F32, isOutput=False)
    out = nc.declare_dram_parameter("out", [1, B], F32, isOutput=True)

    with TileContext(nc) as tc:
        with (
            tc.tile_pool(name="const", bufs=1) as cpool,
            tc.tile_pool(name="xp", bufs=2) as xpool,
            tc.tile_pool(name="psum", bufs=1, space="PSUM") as ppool,
        ):
            # Params on the sync/HWDGE ring so the gpsimd/SWDGE ring starts
            # streaming x immediately.
            wft = cpool.tile([PART, 2 * FREE], F32)
            nc.sync.dma_start(out=wft, in_=wf[:, :])
            # v = |W| * fc1, computed in place over the W half of wft.
            v = wft[:, :FREE]
            nc.scalar.activation(
                out=v, in_=v, func=mybir.ActivationFunctionType.Abs
            )
            nc.vector.tensor_tensor(
                out=v, in0=v, in1=wft[:, FREE:], op=mybir.AluOpType.mult
            )

            ones = cpool.tile([PART, 1], F32)
            nc.vector.memset(ones, 1.0)
            acc = cpool.tile([PART, B], F32)
            scratch = cpool.tile([PART, FREE], F32)

            for g in range(NCHUNKS):
                xt = xpool.tile([PART, CHUNK * FREE], F32, tag="xt")
                nc.gpsimd.dma_start(
                    out=xt, in_=xs[:, g * CHUNK * FREE : (g + 1) * CHUNK * FREE]
                )
                for c in range(CHUNK):
                    b = g * CHUNK + c
                    # Fused multiply + free-dim reduce in one DVE pass:
                    # scratch = (x_b bypass 0) mult v; acc[:, b] = sum(scratch)
                    nc.vector.scalar_tensor_tensor(
                        out=scratch,
                        in0=xt[:, c * FREE : (c + 1) * FREE],
                        scalar=0.0,
                        in1=v,
                        op0=mybir.AluOpType.bypass,
                        op1=mybir.AluOpType.mult,
                        accum_out=acc[:, b : b + 1],
                    )

            ps = ppool.tile([1, B], F32)
            nc.tensor.matmul(out=ps, lhsT=ones, rhs=acc, start=True, stop=True)
            res = cpool.tile([1, B], F32)
            nc.scalar.copy(res, ps)
            nc.sync.dma_start(out=out[:, :], in_=res)
    nc.finalize()
    return nc


def _to_partition_major(flat: np.ndarray) -> np.ndarray:
    """[N, K] row-major -> [PART, N*FREE] where each partition's rows for
    consecutive N are adjacent (N along the middle axis)."""
    n = flat.shape[0]
    padded = np.zeros((n, KPAD), dtype=np.float32)
    padded[:, :K] = flat
    # [n, PART, FREE] -> [PART, n, FREE] -> [PART, n*FREE]
    return np.ascontiguousarray(
        padded.reshape(n, PART, FREE).transpose(1, 0, 2)
    ).reshape(PART, n * FREE)


def make_in_maps(x: np.ndarray, W: np.ndarray, fc1_w: np.ndarray):
    x = np.asarray(x, dtype=np.float32)
    W = np.asarray(W, dtype=np.float32)
    fc1_w = np.asarray(fc1_w, dtype=np.float32)
    fc1_flat = fc1_w.reshape(T, P)
    in_maps = []
    for c in range(NCORES):
        t0 = c * TS
        xs = _to_partition_major(x[:, t0 : t0 + TS, :].reshape(B, K))
        ws = _to_partition_major(W[t0 : t0 + TS, :].reshape(1, K))
        fs = _to_partition_major(fc1_flat[t0 : t0 + TS, :].reshape(1, K))
        in_maps.append({"xs": xs, "wf": np.concatenate([ws, fs], axis=1)})
    return in_maps


def kernel(x, W, fc1_w, fc1_b):
    global LAST_RESULT
    nc = build_program()
    in_maps = make_in_maps(x, W, fc1_w)
    res = run_bass_kernel_spmd(
        nc, in_maps, core_ids=list(range(NCORES)), trace=TRACE
    )
    LAST_RESULT = res
    partial = np.zeros(B, dtype=np.float64)
    for r in res.results:
        partial += r["out"][0].astype(np.float64)
    out = partial.astype(np.float32) + np.float32(np.asarray(fc1_b).reshape(-1)[0])
    return out.reshape(B, 1).astype(np.float32)
